# revision 9
# baseline (speedup 1.0000x reference)
"""Single-head classical attention on 8 TRN2 NeuronCores, K/V-dedup via AllGather.

Problem: B=4, S=2048, D=1024 fp32.
    q = (x @ Wq^T) / sqrt(D); k = x @ Wk^T; v = x @ Wv^T
    out = softmax(q @ k^T) @ v

Sharding: core c handles batch b = c//2 and query-half h = c%2 (1024 query
rows).  Unlike the v1 kernel (which duplicated the full K/V projection on
both cores of a pair), each core projects K/V only for its OWN 1024 keys
(= its own query rows), then the pair exchanges halves with a 2-rank
AllGather ([[0,1],[2,3],[4,5],[6,7]]) that runs on the TOPSP/SDMA
collective engines and overlaps with the Q projection + wait slack.
Key order after the gather is rank order = natural order, so the kernel
output matches the reference ordering exactly.

Host-side staging: inputs are pre-transposed and pre-cast to bf16
(xq = x_b^T own half [D, M]; weights [in, out] = W^T).  This halves input
DMA and removes all on-chip f32->bf16 cast traffic.

On-chip dataflow (matmuls bf16, fp32 PSUM):
    K^T[e,s_own] = matmul(lhsT=WkT[d,e], rhs=Xq[d,s])   -> DRAM, AllGather
    V[s_own,e]   = matmul(lhsT=Xq[d,s],  rhs=WvT[d,e])  -> DRAM, AllGather
    Q^T[e,m]     = matmul(lhsT=WqT[d,e], rhs=Xq[d,m])
    A^T[s,m]     = matmul(lhsT=K^T[e,s], rhs=Q^T[e,m])  (full S after AG)
    P^T[s,m]     = exp(A^T / 32)        (ScalarE; logits ~N(0,1), no max sub)
    z_acc[s,m]  += P^T                  (VectorE partial sums over s-tiles)
    Z[1,m]       = matmul(lhsT=ones[s,1], rhs=z_acc_bf[s,m])
    O[m,e]       = matmul(lhsT=P^T[s,m], rhs=V[s,e]) / Z
"""

import threading

import numpy as np

import concourse.bass as bass
import concourse.tile as tile
from concourse import bacc, mybir
from concourse.bass_utils import run_bass_kernel_spmd

P = 128            # partitions
D = 1024           # embed dim
S = 2048           # seq len (total keys per batch)
M = 1024           # query rows / own keys per core
DT = D // P        # 8  d-tiles (projection contraction)
ET = D // P        # 8  e-tiles
ST = S // P        # 16 s-tiles (full key range)
SHT = M // P       # 8  s-tiles (own half)
MT = M // P        # 8  m-tiles
NF = 512           # matmul free dim (one fp32 PSUM bank)
SCALE = 1.0 / np.sqrt(np.float32(D))  # 1/32

BF16 = mybir.dt.bfloat16
F32 = mybir.dt.float32

GROUPS = [[0, 1], [2, 3], [4, 5], [6, 7]]


def build_attention_core():
    """Build the SPMD Bass graph for one core (same NEFF on all 8 cores)."""
    nc = bacc.Bacc("TRN2", target_bir_lowering=False, debug=False, num_devices=8)

    xq = nc.dram_tensor("xq", [D, M], BF16, kind="ExternalInput")
    wqT = nc.dram_tensor("wqT", [D, D], BF16, kind="ExternalInput")
    wkT = nc.dram_tensor("wkT", [D, D], BF16, kind="ExternalInput")
    wvT = nc.dram_tensor("wvT", [D, D], BF16, kind="ExternalInput")
    out = nc.dram_tensor("out", [M, D], F32, kind="ExternalOutput")

    xq_r = xq.ap().rearrange("(dt p) m -> p dt m", p=P)      # [128, 8, 1024]
    wq_r = wqT.ap().rearrange("(dt p) e -> p dt e", p=P)     # [128, 8, 1024]
    wk_r = wkT.ap().rearrange("(dt p) e -> p dt e", p=P)
    wv_r = wvT.ap().rearrange("(dt p) e -> p dt e", p=P)
    out_r = out.ap().rearrange("(mt p) e -> p mt e", p=P)    # [128, 8, 1024]

    Exp = mybir.ActivationFunctionType.Exp

    with tile.TileContext(nc) as tc:
        with (
            tc.tile_pool(name="persist", bufs=1) as persist,
            # kv staging runs a cast -> DMA -> DRAM pipeline; deep buffering
            # decouples VectorE casts (which gate PSUM recycling and thus
            # TensorE) from staging-DMA completion latency/jitter.
            tc.tile_pool(name="ostage", bufs=8) as ostage,
            tc.tile_pool(name="pp_mm", bufs=6, space="PSUM") as pp_mm,
            tc.tile_pool(name="pp_z", bufs=2, space="PSUM") as pp_z,
            tc.tile_pool(name="dram", bufs=1, space="DRAM") as dram,
        ):
            # ---- persistent bf16 operands ----
            # x dies after the last projection matmul; pT is born in the
            # scores phase -> share one 32KB slot (x uses the first half).
            x_bf = persist.tile([P, DT, M], BF16, tag="xp_share", name="x_bf")
            wq_bf = persist.tile([P, DT, D], BF16, name="wq_bf")
            wk_bf = persist.tile([P, DT, D], BF16, name="wk_bf")
            wv_bf = persist.tile([P, DT, D], BF16, name="wv_bf")
            kT_bf = persist.tile([P, ET, S], BF16, name="kT_bf")
            qT_bf = persist.tile([P, ET, M], BF16, name="qT_bf")
            v_bf = persist.tile([P, ST, D], BF16, name="v_bf")

            ones_bf = persist.tile([P, 1], BF16, name="ones_bf")
            nc.vector.memset(ones_bf[:], 1.0)
            z_acc = persist.tile([P, M], F32, name="z_acc")
            nc.vector.memset(z_acc[:], 0.0)

            # ---- DRAM bounce buffers for the pairwise AllGathers ----
            k_cc_in = dram.tile([D, M], BF16, name="k_cc_in")        # [e, s_own]
            k_cc_out = dram.tile([2, D, M], BF16, name="k_cc_out")
            v_cc_in = dram.tile([SHT, P, D], BF16, name="v_cc_in")   # [st, s, e]
            v_cc_out = dram.tile([2, SHT, P, D], BF16, name="v_cc_out")

            # ---- dummy collective to absorb the CC-stream entry barrier ----
            # The first collective of a NEFF pays a ~25-40us entry cost
            # (rank barrier + mesh stream warm-up).  Triggering a tiny
            # AllGather from the otherwise-idle Scalar queue at t=0 hides
            # that cost under the input-load + K-projection phase, so the
            # real K AllGather starts the moment its data is ready.
            warm_sb = persist.tile([1, 64], BF16, name="warm_sb")
            nc.vector.memset(warm_sb[:], 0.0)
            warm_in = dram.tile([1, 64], BF16, name="warm_in")
            warm_out = dram.tile([2, 64], BF16, name="warm_out")
            nc.scalar.dma_start(warm_in[:], warm_sb[:])
            nc.gpsimd.collective_compute(
                "AllGather",
                mybir.AluOpType.bypass,
                replica_groups=GROUPS,
                ins=[warm_in.opt()],
                outs=[warm_out.opt()],
            )

            # ---- load bf16 inputs (DMA order = PE need order) ----
            for kt in range(DT):
                nc.sync.dma_start(wk_bf[:, kt, :], wk_r[:, kt, :])
                nc.sync.dma_start(x_bf[:, kt, :], xq_r[:, kt, :])
            for kt in range(DT):
                nc.sync.dma_start(wv_bf[:, kt, :], wv_r[:, kt, :])
            for kt in range(DT):
                nc.sync.dma_start(wq_bf[:, kt, :], wq_r[:, kt, :])

            # ---- K^T own half [e, 1024] -> DRAM -> AllGather ----
            for sc in range(M // NF):
                for et in range(ET):
                    ps = pp_mm.tile([P, NF], F32, tag="mm")
                    for kt in range(DT):
                        nc.tensor.matmul(
                            ps[:],
                            lhsT=wk_bf[:, kt, et * P:(et + 1) * P],
                            rhs=x_bf[:, kt, sc * NF:(sc + 1) * NF],
                            start=(kt == 0),
                            stop=(kt == DT - 1),
                        )
                    kst = ostage.tile([P, NF], BF16, tag="kv")
                    nc.vector.tensor_copy(kst[:], ps[:])
                    # scalar queue: gpsimd is reserved for the collectives
                    # (whose completion waits block that queue), and these
                    # staging DMAs gate ostage slot recycling -> VectorE ->
                    # PSUM recycling -> TensorE.
                    nc.scalar.dma_start(
                        k_cc_in[et * P:(et + 1) * P, sc * NF:(sc + 1) * NF],
                        kst[:],
                    )
            nc.gpsimd.collective_compute(
                "AllGather",
                mybir.AluOpType.bypass,
                replica_groups=GROUPS,
                ins=[k_cc_in.opt()],
                outs=[k_cc_out.opt()],
            )

            # ---- V own half [1024, e] -> DRAM -> AllGather ----
            for st in range(SHT):
                for ec in range(D // NF):
                    ps = pp_mm.tile([P, NF], F32, tag="mm")
                    for kt in range(DT):
                        nc.tensor.matmul(
                            ps[:],
                            lhsT=x_bf[:, kt, st * P:(st + 1) * P],
                            rhs=wv_bf[:, kt, ec * NF:(ec + 1) * NF],
                            start=(kt == 0),
                            stop=(kt == DT - 1),
                        )
                    vst = ostage.tile([P, NF], BF16, tag="kv")
                    nc.vector.tensor_copy(vst[:], ps[:])
                    # sync queue, NOT gpsimd: the gpsimd queue is blocked on
                    # the K AllGather completion, and these staging DMAs gate
                    # the ostage slot recycling that feeds the VectorE casts
                    # (and through PSUM recycling, the TensorE itself).
                    nc.sync.dma_start(
                        v_cc_in[st, :, ec * NF:(ec + 1) * NF], vst[:]
                    )
            nc.gpsimd.collective_compute(
                "AllGather",
                mybir.AluOpType.bypass,
                replica_groups=GROUPS,
                ins=[v_cc_in.opt()],
                outs=[v_cc_out.opt()],
            )

            # ---- Q^T [e, m] (stays on-chip) ----
            for et in range(ET):
                for mc in range(M // NF):
                    ps = pp_mm.tile([P, NF], F32, tag="mm")
                    for kt in range(DT):
                        nc.tensor.matmul(
                            ps[:],
                            lhsT=wq_bf[:, kt, et * P:(et + 1) * P],
                            rhs=x_bf[:, kt, mc * NF:(mc + 1) * NF],
                            start=(kt == 0),
                            stop=(kt == DT - 1),
                        )
                    nc.vector.tensor_copy(qT_bf[:, et, mc * NF:(mc + 1) * NF], ps[:])

            # ---- gather results back: rank order = natural key order ----
            # k_back on the scalar queue (idle until the A-phase exps, which
            # start only after the first A matmul group anyway); v_back on
            # sync (blocks it until the V-AG completes ~105us, well before
            # the z-bounce/output stores need it).
            for r in range(2):
                nc.scalar.dma_start(
                    kT_bf[:, :, r * M:(r + 1) * M],
                    k_cc_out[r].rearrange("(et p) s -> p et s", p=P),
                )
            for r in range(2):
                nc.sync.dma_start(
                    v_bf[:, r * SHT:(r + 1) * SHT, :],
                    v_cc_out[r].rearrange("st p e -> p st e"),
                )

            # ---- scores: A^T = K @ Q^T, P^T = exp(A^T/32), z_acc += P^T ----
            pT_bf = persist.tile([P, ST, M], BF16, tag="xp_share", name="pT_bf")

            for st in range(ST):
                for mc in range(M // NF):
                    ps_a = pp_mm.tile([P, NF], F32, tag="mm")
                    for et in range(ET):
                        nc.tensor.matmul(
                            ps_a[:],
                            lhsT=kT_bf[:, et, st * P:(st + 1) * P],
                            rhs=qT_bf[:, et, mc * NF:(mc + 1) * NF],
                            start=(et == 0),
                            stop=(et == ET - 1),
                        )
                    nc.scalar.activation(
                        out=pT_bf[:, st, mc * NF:(mc + 1) * NF],
                        in_=ps_a[:],
                        func=Exp,
                        scale=float(SCALE),
                    )
                    nc.vector.tensor_add(
                        out=z_acc[:, mc * NF:(mc + 1) * NF],
                        in0=z_acc[:, mc * NF:(mc + 1) * NF],
                        in1=pT_bf[:, st, mc * NF:(mc + 1) * NF],
                    )

            # ---- softmax denominators: partition-reduce z_acc via ones-mm,
            # bounce [1, M] through DRAM to get [128, MT] columns, recip ----
            z_bf = persist.tile([P, M], BF16, name="z_bf")
            nc.vector.tensor_copy(z_bf[:], z_acc[:])
            z_row = persist.tile([1, M], F32, name="z_row")
            for mc in range(M // NF):
                ps_z = pp_z.tile([1, NF], F32, tag="z")
                nc.tensor.matmul(
                    ps_z[:],
                    lhsT=ones_bf[:],
                    rhs=z_bf[:, mc * NF:(mc + 1) * NF],
                    start=True,
                    stop=True,
                )
                nc.vector.tensor_copy(z_row[:, mc * NF:(mc + 1) * NF], ps_z[:])
            z_dram = dram.tile([1, M], F32, name="z_dram")
            nc.sync.dma_start(z_dram[:], z_row[:])
            z_col = persist.tile([P, MT], F32, name="z_col")
            nc.sync.dma_start(
                z_col[:], z_dram[0, :].rearrange("(t p) -> p t", p=P)
            )
            z_recip = persist.tile([P, MT], F32, name="z_recip")
            nc.vector.reciprocal(z_recip[:], z_col[:])

            # ---- O = (P^T)^T @ V, scaled by 1/Z ----
            for mt in range(MT):
                for ec in range(D // NF):
                    ps_o = pp_mm.tile([P, NF], F32, tag="mm")
                    for st in range(ST):
                        nc.tensor.matmul(
                            ps_o[:],
                            lhsT=pT_bf[:, st, mt * P:(mt + 1) * P],
                            rhs=v_bf[:, st, ec * NF:(ec + 1) * NF],
                            start=(st == 0),
                            stop=(st == ST - 1),
                        )
                    o_t = ostage.tile([P, NF], F32, tag="o")
                    nc.vector.tensor_scalar_mul(
                        o_t[:], ps_o[:], z_recip[:, mt:mt + 1]
                    )
                    nc.sync.dma_start(out_r[:, mt, ec * NF:(ec + 1) * NF], o_t[:])

    nc.compile()
    return nc


_nc_lock = threading.Lock()
_nc_cache = []


def _get_nc():
    with _nc_lock:
        if not _nc_cache:
            _nc_cache.append(build_attention_core())
        return _nc_cache[0]


def _make_in_maps(inputs, w_q, w_k, w_v):
    import ml_dtypes

    bf = ml_dtypes.bfloat16
    wqT = np.ascontiguousarray(np.asarray(w_q, dtype=np.float32).T).astype(bf)
    wkT = np.ascontiguousarray(np.asarray(w_k, dtype=np.float32).T).astype(bf)
    wvT = np.ascontiguousarray(np.asarray(w_v, dtype=np.float32).T).astype(bf)
    in_maps = []
    for core in range(8):
        b, half = core // 2, core % 2
        xq = np.asarray(inputs[b][half * M:(half + 1) * M, :], dtype=np.float32)
        in_maps.append(
            {
                "xq": np.ascontiguousarray(xq.T).astype(bf),
                "wqT": wqT,
                "wkT": wkT,
                "wvT": wvT,
            }
        )
    return in_maps


def run(inputs, w_q, w_k, w_v, **run_kwargs):
    """Run the 8-core SPMD kernel; returns (full_output, BassKernelResults)."""
    nc = _get_nc()
    in_maps = _make_in_maps(inputs, w_q, w_k, w_v)
    res = run_bass_kernel_spmd(nc, in_maps, core_ids=list(range(8)), **run_kwargs)
    full = np.empty((4, S, D), dtype=np.float32)
    for core in range(8):
        b, half = core // 2, core % 2
        full[b, half * M:(half + 1) * M, :] = res.results[core]["out"]
    return full, res


def kernel(**inputs) -> np.ndarray:
    out, _ = run(inputs["inputs"], inputs["w_q"], inputs["w_k"], inputs["w_v"])
    return out


# revision 13
# speedup vs baseline: 1.0632x; 1.0632x over previous
"""Single-head classical attention on 8 TRN2 NeuronCores, K/V-dedup via AllGather.

Problem: B=4, S=2048, D=1024 fp32.
    q = (x @ Wq^T) / sqrt(D); k = x @ Wk^T; v = x @ Wv^T
    out = softmax(q @ k^T) @ v

Sharding: core c handles batch b = c//2 and query-half h = c%2 (1024 query
rows).  Unlike the v1 kernel (which duplicated the full K/V projection on
both cores of a pair), each core projects K/V only for its OWN 1024 keys
(= its own query rows), then the pair exchanges halves with a 2-rank
AllGather ([[0,1],[2,3],[4,5],[6,7]]) that runs on the TOPSP/SDMA
collective engines and overlaps with the Q projection + wait slack.
Key order after the gather is rank order = natural order, so the kernel
output matches the reference ordering exactly.

Host-side staging: inputs are pre-transposed and pre-cast to bf16
(xq = x_b^T own half [D, M]; weights [in, out] = W^T).  This halves input
DMA and removes all on-chip f32->bf16 cast traffic.

On-chip dataflow (matmuls bf16, fp32 PSUM):
    K^T[e,s_own] = matmul(lhsT=WkT[d,e], rhs=Xq[d,s])   -> DRAM, AllGather
    V[s_own,e]   = matmul(lhsT=Xq[d,s],  rhs=WvT[d,e])  -> DRAM, AllGather
    Q^T[e,m]     = matmul(lhsT=WqT[d,e], rhs=Xq[d,m])
    A^T[s,m]     = matmul(lhsT=K^T[e,s], rhs=Q^T[e,m])  (full S after AG)
    P^T[s,m]     = exp(A^T / 32)        (ScalarE; logits ~N(0,1), no max sub)
    z_acc[s,m]  += P^T                  (VectorE partial sums over s-tiles)
    Z[1,m]       = matmul(lhsT=ones[s,1], rhs=z_acc_bf[s,m])
    O[m,e]       = matmul(lhsT=P^T[s,m], rhs=V[s,e]) / Z
"""

import threading

import numpy as np

import concourse.bass as bass
import concourse.tile as tile
from concourse import bacc, mybir
from concourse.bass_utils import run_bass_kernel_spmd

P = 128            # partitions
D = 1024           # embed dim
S = 2048           # seq len (total keys per batch)
M = 1024           # query rows / own keys per core
DT = D // P        # 8  d-tiles (projection contraction)
ET = D // P        # 8  e-tiles
ST = S // P        # 16 s-tiles (full key range)
SHT = M // P       # 8  s-tiles (own half)
MT = M // P        # 8  m-tiles
NF = 512           # matmul free dim (one fp32 PSUM bank)
SCALE = 1.0 / np.sqrt(np.float32(D))  # 1/32

BF16 = mybir.dt.bfloat16
F32 = mybir.dt.float32

GROUPS = [[0, 1], [2, 3], [4, 5], [6, 7]]


def build_attention_core():
    """Build the SPMD Bass graph for one core (same NEFF on all 8 cores)."""
    nc = bacc.Bacc("TRN2", target_bir_lowering=False, debug=False, num_devices=8)

    xq = nc.dram_tensor("xq", [D, M], BF16, kind="ExternalInput")
    wqT = nc.dram_tensor("wqT", [D, D], BF16, kind="ExternalInput")
    wkT = nc.dram_tensor("wkT", [D, D], BF16, kind="ExternalInput")
    wvT = nc.dram_tensor("wvT", [D, D], BF16, kind="ExternalInput")
    out = nc.dram_tensor("out", [M, D], F32, kind="ExternalOutput")

    xq_r = xq.ap().rearrange("(dt p) m -> p dt m", p=P)      # [128, 8, 1024]
    wq_r = wqT.ap().rearrange("(dt p) e -> p dt e", p=P)     # [128, 8, 1024]
    wk_r = wkT.ap().rearrange("(dt p) e -> p dt e", p=P)
    wv_r = wvT.ap().rearrange("(dt p) e -> p dt e", p=P)
    out_r = out.ap().rearrange("(mt p) e -> p mt e", p=P)    # [128, 8, 1024]

    Exp = mybir.ActivationFunctionType.Exp

    with tile.TileContext(nc) as tc:
        with (
            tc.tile_pool(name="persist", bufs=1) as persist,
            # kv staging runs a cast -> DMA -> DRAM pipeline; deep buffering
            # decouples VectorE casts (which gate PSUM recycling and thus
            # TensorE) from staging-DMA completion latency/jitter.
            tc.tile_pool(name="ostage", bufs=8) as ostage,
            tc.tile_pool(name="pp_mm", bufs=6, space="PSUM") as pp_mm,
            tc.tile_pool(name="pp_z", bufs=2, space="PSUM") as pp_z,
            tc.tile_pool(name="dram", bufs=1, space="DRAM") as dram,
        ):
            # ---- persistent bf16 operands ----
            # x dies after the last projection matmul; pT is born in the
            # scores phase -> share one 32KB slot (x uses the first half).
            x_bf = persist.tile([P, DT, M], BF16, tag="xp_share", name="x_bf")
            wq_bf = persist.tile([P, DT, D], BF16, name="wq_bf")
            wk_bf = persist.tile([P, DT, D], BF16, name="wk_bf")
            wv_bf = persist.tile([P, DT, D], BF16, name="wv_bf")
            kT_bf = persist.tile([P, ET, S], BF16, name="kT_bf")
            qT_bf = persist.tile([P, ET, M], BF16, name="qT_bf")
            v_bf = persist.tile([P, ST, D], BF16, name="v_bf")

            ones_bf = persist.tile([P, 1], BF16, name="ones_bf")
            nc.vector.memset(ones_bf[:], 1.0)
            z_acc = persist.tile([P, M], F32, name="z_acc")
            nc.vector.memset(z_acc[:], 0.0)

            # ---- DRAM bounce buffers for the pairwise AllGathers ----
            # K is exchanged in two 1MB chunks (s-columns 0:512 / 512:1024)
            # so the first chunk is back in SBUF well before the PE reaches
            # the scores phase, even on a full-speed (2.4GHz) run.
            k_cc_in = [
                dram.tile([D, NF], BF16, name=f"k_cc_in{c}") for c in range(2)
            ]
            k_cc_out = [
                dram.tile([2, D, NF], BF16, name=f"k_cc_out{c}") for c in range(2)
            ]
            v_cc_in = dram.tile([SHT, P, D], BF16, name="v_cc_in")   # [st, s, e]
            v_cc_out = dram.tile([2, SHT, P, D], BF16, name="v_cc_out")

            # ---- dummy collective to absorb the CC-stream entry barrier ----
            # The first collective of a NEFF pays a ~25-40us entry cost
            # (rank barrier + mesh stream warm-up).  Triggering a tiny
            # AllGather from the otherwise-idle Scalar queue at t=0 hides
            # that cost under the input-load + K-projection phase, so the
            # real K AllGather starts the moment its data is ready.
            warm_sb = persist.tile([1, 64], BF16, name="warm_sb")
            nc.vector.memset(warm_sb[:], 0.0)
            warm_in = dram.tile([1, 64], BF16, name="warm_in")
            warm_out = dram.tile([2, 64], BF16, name="warm_out")
            nc.scalar.dma_start(warm_in[:], warm_sb[:])
            nc.gpsimd.collective_compute(
                "AllGather",
                mybir.AluOpType.bypass,
                replica_groups=GROUPS,
                ins=[warm_in.opt()],
                outs=[warm_out.opt()],
            )

            # ---- load bf16 inputs (DMA order = PE need order) ----
            # x on the scalar queue so it streams in parallel with wk (sync)
            # and the first K matmul group is ready ~2us sooner.
            for kt in range(DT):
                nc.sync.dma_start(wk_bf[:, kt, :], wk_r[:, kt, :])
                nc.scalar.dma_start(x_bf[:, kt, :], xq_r[:, kt, :])
            for kt in range(DT):
                nc.sync.dma_start(wv_bf[:, kt, :], wv_r[:, kt, :])
            for kt in range(DT):
                nc.sync.dma_start(wq_bf[:, kt, :], wq_r[:, kt, :])

            # ---- K^T own half [e, 1024] -> DRAM -> AllGather (2 chunks) ----
            for sc in range(M // NF):
                for et in range(ET):
                    ps = pp_mm.tile([P, NF], F32, tag="mm")
                    for kt in range(DT):
                        nc.tensor.matmul(
                            ps[:],
                            lhsT=wk_bf[:, kt, et * P:(et + 1) * P],
                            rhs=x_bf[:, kt, sc * NF:(sc + 1) * NF],
                            start=(kt == 0),
                            stop=(kt == DT - 1),
                        )
                    kst = ostage.tile([P, NF], BF16, tag="kv")
                    nc.vector.tensor_copy(kst[:], ps[:])
                    # scalar queue: gpsimd is reserved for the collectives
                    # (whose completion waits block that queue), and these
                    # staging DMAs gate ostage slot recycling -> VectorE ->
                    # PSUM recycling -> TensorE.
                    nc.scalar.dma_start(
                        k_cc_in[sc][et * P:(et + 1) * P, :], kst[:]
                    )
                nc.gpsimd.collective_compute(
                    "AllGather",
                    mybir.AluOpType.bypass,
                    replica_groups=GROUPS,
                    ins=[k_cc_in[sc].opt()],
                    outs=[k_cc_out[sc].opt()],
                )

            # ---- V own half [1024, e] -> DRAM -> AllGather ----
            for st in range(SHT):
                for ec in range(D // NF):
                    ps = pp_mm.tile([P, NF], F32, tag="mm")
                    for kt in range(DT):
                        nc.tensor.matmul(
                            ps[:],
                            lhsT=x_bf[:, kt, st * P:(st + 1) * P],
                            rhs=wv_bf[:, kt, ec * NF:(ec + 1) * NF],
                            start=(kt == 0),
                            stop=(kt == DT - 1),
                        )
                    vst = ostage.tile([P, NF], BF16, tag="kv")
                    nc.vector.tensor_copy(vst[:], ps[:])
                    # sync queue, NOT gpsimd: the gpsimd queue is blocked on
                    # the K AllGather completion, and these staging DMAs gate
                    # the ostage slot recycling that feeds the VectorE casts
                    # (and through PSUM recycling, the TensorE itself).
                    nc.sync.dma_start(
                        v_cc_in[st, :, ec * NF:(ec + 1) * NF], vst[:]
                    )
            nc.gpsimd.collective_compute(
                "AllGather",
                mybir.AluOpType.bypass,
                replica_groups=GROUPS,
                ins=[v_cc_in.opt()],
                outs=[v_cc_out.opt()],
            )

            # ---- Q^T [e, m] (stays on-chip) ----
            for et in range(ET):
                for mc in range(M // NF):
                    ps = pp_mm.tile([P, NF], F32, tag="mm")
                    for kt in range(DT):
                        nc.tensor.matmul(
                            ps[:],
                            lhsT=wq_bf[:, kt, et * P:(et + 1) * P],
                            rhs=x_bf[:, kt, mc * NF:(mc + 1) * NF],
                            start=(kt == 0),
                            stop=(kt == DT - 1),
                        )
                    nc.vector.tensor_copy(qT_bf[:, et, mc * NF:(mc + 1) * NF], ps[:])

            # ---- gather results back: rank order = natural key order ----
            # k_back on the scalar queue (idle until the A-phase exps, which
            # start only after the first A matmul group anyway); v_back on
            # sync (blocks it until the V-AG completes, well before the
            # z-bounce/output stores need it).  Chunk c region r covers
            # kT s-columns [r*1024 + c*512, +512).
            for c in range(2):
                for r in range(2):
                    nc.scalar.dma_start(
                        kT_bf[:, :, r * M + c * NF:r * M + (c + 1) * NF],
                        k_cc_out[c][r].rearrange("(et p) s -> p et s", p=P),
                    )
            for r in range(2):
                nc.sync.dma_start(
                    v_bf[:, r * SHT:(r + 1) * SHT, :],
                    v_cc_out[r].rearrange("st p e -> p st e"),
                )

            # ---- scores: A^T = K @ Q^T, P^T = exp(A^T/32), z_acc += P^T ----
            pT_bf = persist.tile([P, ST, M], BF16, tag="xp_share", name="pT_bf")

            for st in range(ST):
                for mc in range(M // NF):
                    ps_a = pp_mm.tile([P, NF], F32, tag="mm")
                    for et in range(ET):
                        nc.tensor.matmul(
                            ps_a[:],
                            lhsT=kT_bf[:, et, st * P:(st + 1) * P],
                            rhs=qT_bf[:, et, mc * NF:(mc + 1) * NF],
                            start=(et == 0),
                            stop=(et == ET - 1),
                        )
                    nc.scalar.activation(
                        out=pT_bf[:, st, mc * NF:(mc + 1) * NF],
                        in_=ps_a[:],
                        func=Exp,
                        scale=float(SCALE),
                    )
                    nc.vector.tensor_add(
                        out=z_acc[:, mc * NF:(mc + 1) * NF],
                        in0=z_acc[:, mc * NF:(mc + 1) * NF],
                        in1=pT_bf[:, st, mc * NF:(mc + 1) * NF],
                    )

            # ---- softmax denominators: partition-reduce z_acc via ones-mm,
            # bounce [1, M] through DRAM to get [128, MT] columns, recip ----
            z_bf = persist.tile([P, M], BF16, name="z_bf")
            nc.vector.tensor_copy(z_bf[:], z_acc[:])
            z_row = persist.tile([1, M], F32, name="z_row")
            for mc in range(M // NF):
                ps_z = pp_z.tile([1, NF], F32, tag="z")
                nc.tensor.matmul(
                    ps_z[:],
                    lhsT=ones_bf[:],
                    rhs=z_bf[:, mc * NF:(mc + 1) * NF],
                    start=True,
                    stop=True,
                )
                nc.vector.tensor_copy(z_row[:, mc * NF:(mc + 1) * NF], ps_z[:])
            z_dram = dram.tile([1, M], F32, name="z_dram")
            nc.sync.dma_start(z_dram[:], z_row[:])
            z_col = persist.tile([P, MT], F32, name="z_col")
            nc.sync.dma_start(
                z_col[:], z_dram[0, :].rearrange("(t p) -> p t", p=P)
            )
            z_recip = persist.tile([P, MT], F32, name="z_recip")
            nc.vector.reciprocal(z_recip[:], z_col[:])

            # ---- O = (P^T)^T @ V, scaled by 1/Z ----
            for mt in range(MT):
                for ec in range(D // NF):
                    ps_o = pp_mm.tile([P, NF], F32, tag="mm")
                    for st in range(ST):
                        nc.tensor.matmul(
                            ps_o[:],
                            lhsT=pT_bf[:, st, mt * P:(mt + 1) * P],
                            rhs=v_bf[:, st, ec * NF:(ec + 1) * NF],
                            start=(st == 0),
                            stop=(st == ST - 1),
                        )
                    o_t = ostage.tile([P, NF], F32, tag="o")
                    nc.vector.tensor_scalar_mul(
                        o_t[:], ps_o[:], z_recip[:, mt:mt + 1]
                    )
                    nc.sync.dma_start(out_r[:, mt, ec * NF:(ec + 1) * NF], o_t[:])

    nc.compile()
    return nc


_nc_lock = threading.Lock()
_nc_cache = []


def _get_nc():
    with _nc_lock:
        if not _nc_cache:
            _nc_cache.append(build_attention_core())
        return _nc_cache[0]


def _make_in_maps(inputs, w_q, w_k, w_v):
    import ml_dtypes

    bf = ml_dtypes.bfloat16
    wqT = np.ascontiguousarray(np.asarray(w_q, dtype=np.float32).T).astype(bf)
    wkT = np.ascontiguousarray(np.asarray(w_k, dtype=np.float32).T).astype(bf)
    wvT = np.ascontiguousarray(np.asarray(w_v, dtype=np.float32).T).astype(bf)
    in_maps = []
    for core in range(8):
        b, half = core // 2, core % 2
        xq = np.asarray(inputs[b][half * M:(half + 1) * M, :], dtype=np.float32)
        in_maps.append(
            {
                "xq": np.ascontiguousarray(xq.T).astype(bf),
                "wqT": wqT,
                "wkT": wkT,
                "wvT": wvT,
            }
        )
    return in_maps


def run(inputs, w_q, w_k, w_v, **run_kwargs):
    """Run the 8-core SPMD kernel; returns (full_output, BassKernelResults)."""
    nc = _get_nc()
    in_maps = _make_in_maps(inputs, w_q, w_k, w_v)
    res = run_bass_kernel_spmd(nc, in_maps, core_ids=list(range(8)), **run_kwargs)
    full = np.empty((4, S, D), dtype=np.float32)
    for core in range(8):
        b, half = core // 2, core % 2
        full[b, half * M:(half + 1) * M, :] = res.results[core]["out"]
    return full, res


def kernel(**inputs) -> np.ndarray:
    out, _ = run(inputs["inputs"], inputs["w_q"], inputs["w_k"], inputs["w_v"])
    return out


# revision 20
# speedup vs baseline: 1.1112x; 1.0451x over previous
"""Single-head classical attention on 8 TRN2 NeuronCores, K/V-dedup via AllGather.

Problem: B=4, S=2048, D=1024 fp32.
    q = (x @ Wq^T) / sqrt(D); k = x @ Wk^T; v = x @ Wv^T
    out = softmax(q @ k^T) @ v

Sharding: core c handles batch b = c//2 and query-half h = c%2 (1024 query
rows).  Unlike the v1 kernel (which duplicated the full K/V projection on
both cores of a pair), each core projects K/V only for its OWN 1024 keys
(= its own query rows), then the pair exchanges halves with a 2-rank
AllGather ([[0,1],[2,3],[4,5],[6,7]]) that runs on the TOPSP/SDMA
collective engines and overlaps with the Q projection + wait slack.
Key order after the gather is rank order = natural order, so the kernel
output matches the reference ordering exactly.

Host-side staging: inputs are pre-transposed and pre-cast to bf16
(xq = x_b^T own half [D, M]; weights [in, out] = W^T).  This halves input
DMA and removes all on-chip f32->bf16 cast traffic.

On-chip dataflow (matmuls bf16, fp32 PSUM):
    K^T[e,s_own] = matmul(lhsT=WkT[d,e], rhs=Xq[d,s])   -> DRAM, AllGather
    V[s_own,e]   = matmul(lhsT=Xq[d,s],  rhs=WvT[d,e])  -> DRAM, AllGather
    Q^T[e,m]     = matmul(lhsT=WqT[d,e], rhs=Xq[d,m])
    A^T[s,m]     = matmul(lhsT=K^T[e,s], rhs=Q^T[e,m])  (full S after AG)
    P^T[s,m]     = exp(A^T / 32)        (ScalarE; logits ~N(0,1), no max sub)
    z_acc[s,m]  += P^T                  (VectorE partial sums over s-tiles)
    Z[1,m]       = matmul(lhsT=ones[s,1], rhs=z_acc_bf[s,m])
    O[m,e]       = matmul(lhsT=P^T[s,m], rhs=V[s,e]) / Z
"""

import threading

import numpy as np

import concourse.bass as bass
import concourse.tile as tile
from concourse import bacc, mybir
from concourse.bass_utils import run_bass_kernel_spmd

P = 128            # partitions
D = 1024           # embed dim
S = 2048           # seq len (total keys per batch)
M = 1024           # query rows / own keys per core
DT = D // P        # 8  d-tiles (projection contraction)
ET = D // P        # 8  e-tiles
ST = S // P        # 16 s-tiles (full key range)
SHT = M // P       # 8  s-tiles (own half)
MT = M // P        # 8  m-tiles
NF = 512           # matmul free dim (one fp32 PSUM bank)
SCALE = 1.0 / np.sqrt(np.float32(D))  # 1/32

BF16 = mybir.dt.bfloat16
F32 = mybir.dt.float32

GROUPS = [[0, 1], [2, 3], [4, 5], [6, 7]]


def build_attention_core():
    """Build the SPMD Bass graph for one core (same NEFF on all 8 cores)."""
    nc = bacc.Bacc("TRN2", target_bir_lowering=False, debug=False, num_devices=8)

    xq = nc.dram_tensor("xq", [D, M], BF16, kind="ExternalInput")
    wqT = nc.dram_tensor("wqT", [D, D], BF16, kind="ExternalInput")
    wkT = nc.dram_tensor("wkT", [D, D], BF16, kind="ExternalInput")
    wvT = nc.dram_tensor("wvT", [D, D], BF16, kind="ExternalInput")
    out = nc.dram_tensor("out", [M, D], F32, kind="ExternalOutput")

    xq_r = xq.ap().rearrange("(dt p) m -> p dt m", p=P)      # [128, 8, 1024]
    wq_r = wqT.ap().rearrange("(dt p) e -> p dt e", p=P)     # [128, 8, 1024]
    wk_r = wkT.ap().rearrange("(dt p) e -> p dt e", p=P)
    wv_r = wvT.ap().rearrange("(dt p) e -> p dt e", p=P)
    out_r = out.ap().rearrange("(mt p) e -> p mt e", p=P)    # [128, 8, 1024]

    Exp = mybir.ActivationFunctionType.Exp

    with tile.TileContext(nc) as tc:
        with (
            tc.tile_pool(name="persist", bufs=1) as persist,
            # kv staging runs a cast -> DMA -> DRAM pipeline; deep buffering
            # decouples VectorE casts (which gate PSUM recycling and thus
            # TensorE) from staging-DMA completion latency/jitter.
            tc.tile_pool(name="ostage", bufs=8) as ostage,
            tc.tile_pool(name="pp_mm", bufs=6, space="PSUM") as pp_mm,
            tc.tile_pool(name="pp_z", bufs=2, space="PSUM") as pp_z,
            tc.tile_pool(name="dram", bufs=1, space="DRAM") as dram,
        ):
            # ---- persistent bf16 operands ----
            # x dies after the last projection matmul; pT is born in the
            # scores phase -> share one 32KB slot (x uses the first half).
            x_bf = persist.tile([P, DT, M], BF16, tag="xp_share", name="x_bf")
            wq_bf = persist.tile([P, DT, D], BF16, name="wq_bf")
            wk_bf = persist.tile([P, DT, D], BF16, name="wk_bf")
            wv_bf = persist.tile([P, DT, D], BF16, name="wv_bf")
            qT_bf = persist.tile([P, ET, M], BF16, name="qT_bf")
            # K^T / V live in per-(chunk, rank) tiles so each gather-back DMA
            # is one whole-tile transfer with 8-16KB contiguous lines on both
            # sides (strided 1KB-line DMAs measured ~3x slower).  s-tile st
            # maps to kT4[(st % 8) // 4][st // 8] col (st % 4), v2[st // 8]
            # row (st % 8).
            kT4 = [
                [
                    persist.tile([P, ET, NF], BF16, name=f"kT_c{c}r{r}")
                    for r in range(2)
                ]
                for c in range(2)
            ]
            v2 = [
                persist.tile([P, SHT, D], BF16, name=f"v_r{r}") for r in range(2)
            ]

            ones_bf = persist.tile([P, 1], BF16, name="ones_bf")
            nc.vector.memset(ones_bf[:], 1.0)
            z_acc = persist.tile([P, M], F32, name="z_acc")
            nc.vector.memset(z_acc[:], 0.0)

            # ---- DRAM bounce buffers for the pairwise AllGathers ----
            # K is exchanged in two 1MB chunks (s-columns 0:512 / 512:1024)
            # so the first chunk is back in SBUF well before the PE reaches
            # the scores phase, even on a full-speed (2.4GHz) run.
            k_cc_in = [
                dram.tile([P, ET, NF], BF16, name=f"k_cc_in{c}") for c in range(2)
            ]
            k_cc_out = [
                dram.tile([2, P, ET, NF], BF16, name=f"k_cc_out{c}")
                for c in range(2)
            ]
            v_cc_in = dram.tile([P, SHT, D], BF16, name="v_cc_in")   # [p, st, e]
            v_cc_out = dram.tile([2, P, SHT, D], BF16, name="v_cc_out")

            # ---- dummy collective to absorb the CC-stream entry barrier ----
            # The first collective of a NEFF pays a ~25-40us entry cost
            # (rank barrier + mesh stream warm-up).  Triggering a tiny
            # AllGather from the otherwise-idle Scalar queue at t=0 hides
            # that cost under the input-load + K-projection phase, so the
            # real K AllGather starts the moment its data is ready.
            warm_sb = persist.tile([1, 64], BF16, name="warm_sb")
            nc.vector.memset(warm_sb[:], 0.0)
            warm_in = dram.tile([1, 64], BF16, name="warm_in")
            warm_out = dram.tile([2, 64], BF16, name="warm_out")
            nc.scalar.dma_start(warm_in[:], warm_sb[:])
            nc.gpsimd.collective_compute(
                "AllGather",
                mybir.AluOpType.bypass,
                replica_groups=GROUPS,
                ins=[warm_in.opt()],
                outs=[warm_out.opt()],
            )

            # ---- load bf16 inputs (DMA order = PE need order) ----
            # x on the scalar queue so it streams in parallel with wk (sync)
            # and the first K matmul group is ready ~2us sooner.
            for kt in range(DT):
                nc.sync.dma_start(wk_bf[:, kt, :], wk_r[:, kt, :])
                nc.scalar.dma_start(x_bf[:, kt, :], xq_r[:, kt, :])
            for kt in range(DT):
                nc.sync.dma_start(wv_bf[:, kt, :], wv_r[:, kt, :])
            for kt in range(DT):
                nc.sync.dma_start(wq_bf[:, kt, :], wq_r[:, kt, :])

            # ---- K^T own half [e, 1024] -> DRAM -> AllGather (2 chunks) ----
            for sc in range(M // NF):
                for et in range(ET):
                    ps = pp_mm.tile([P, NF], F32, tag="mm")
                    for kt in range(DT):
                        nc.tensor.matmul(
                            ps[:],
                            lhsT=wk_bf[:, kt, et * P:(et + 1) * P],
                            rhs=x_bf[:, kt, sc * NF:(sc + 1) * NF],
                            start=(kt == 0),
                            stop=(kt == DT - 1),
                        )
                    kst = ostage.tile([P, NF], BF16, tag="kv")
                    nc.vector.tensor_copy(kst[:], ps[:])
                    # scalar queue: gpsimd is reserved for the collectives
                    # (whose completion waits block that queue), and these
                    # staging DMAs gate ostage slot recycling -> VectorE ->
                    # PSUM recycling -> TensorE.
                    nc.scalar.dma_start(k_cc_in[sc][:, et, :], kst[:])
                nc.gpsimd.collective_compute(
                    "AllGather",
                    mybir.AluOpType.bypass,
                    replica_groups=GROUPS,
                    ins=[k_cc_in[sc].opt()],
                    outs=[k_cc_out[sc].opt()],
                )

            # ---- V own half [1024, e] -> DRAM -> AllGather ----
            for st in range(SHT):
                for ec in range(D // NF):
                    ps = pp_mm.tile([P, NF], F32, tag="mm")
                    for kt in range(DT):
                        nc.tensor.matmul(
                            ps[:],
                            lhsT=x_bf[:, kt, st * P:(st + 1) * P],
                            rhs=wv_bf[:, kt, ec * NF:(ec + 1) * NF],
                            start=(kt == 0),
                            stop=(kt == DT - 1),
                        )
                    vst = ostage.tile([P, NF], BF16, tag="kv")
                    nc.vector.tensor_copy(vst[:], ps[:])
                    # sync queue, NOT gpsimd: the gpsimd queue is blocked on
                    # the K AllGather completion, and these staging DMAs gate
                    # the ostage slot recycling that feeds the VectorE casts
                    # (and through PSUM recycling, the TensorE itself).
                    nc.sync.dma_start(
                        v_cc_in[:, st, ec * NF:(ec + 1) * NF], vst[:]
                    )
            nc.gpsimd.collective_compute(
                "AllGather",
                mybir.AluOpType.bypass,
                replica_groups=GROUPS,
                ins=[v_cc_in.opt()],
                outs=[v_cc_out.opt()],
            )

            # ---- Q^T [e, m] (stays on-chip) ----
            for et in range(ET):
                for mc in range(M // NF):
                    ps = pp_mm.tile([P, NF], F32, tag="mm")
                    for kt in range(DT):
                        nc.tensor.matmul(
                            ps[:],
                            lhsT=wq_bf[:, kt, et * P:(et + 1) * P],
                            rhs=x_bf[:, kt, mc * NF:(mc + 1) * NF],
                            start=(kt == 0),
                            stop=(kt == DT - 1),
                        )
                    nc.vector.tensor_copy(qT_bf[:, et, mc * NF:(mc + 1) * NF], ps[:])

            # ---- gather results back: rank order = natural key order ----
            # k_back on the scalar queue (idle until the A-phase exps, which
            # start only after the first A matmul group anyway); v_back on
            # gpsimd, which is empty once the V AllGather it waits on has
            # completed (on sync it was scheduled behind the z-bounce DMA,
            # which is only ready at the END of the scores phase -> 13us
            # TensorE stall in the O phase).
            for c in range(2):
                for r in range(2):
                    nc.scalar.dma_start(kT4[c][r][:], k_cc_out[c][r])
            for r in range(2):
                nc.gpsimd.dma_start(v2[r][:], v_cc_out[r])

            # ---- scores: A^T = K @ Q^T, P^T = exp(A^T/32), z_acc += P^T ----
            pT_bf = persist.tile([P, ST, M], BF16, tag="xp_share", name="pT_bf")

            for st in range(ST):
                k_t = kT4[(st % 8) // 4][st // 8]
                off = st % 4
                for mc in range(M // NF):
                    ps_a = pp_mm.tile([P, NF], F32, tag="mm")
                    for et in range(ET):
                        nc.tensor.matmul(
                            ps_a[:],
                            lhsT=k_t[:, et, off * P:(off + 1) * P],
                            rhs=qT_bf[:, et, mc * NF:(mc + 1) * NF],
                            start=(et == 0),
                            stop=(et == ET - 1),
                        )
                    nc.scalar.activation(
                        out=pT_bf[:, st, mc * NF:(mc + 1) * NF],
                        in_=ps_a[:],
                        func=Exp,
                        scale=float(SCALE),
                    )
                    nc.vector.tensor_add(
                        out=z_acc[:, mc * NF:(mc + 1) * NF],
                        in0=z_acc[:, mc * NF:(mc + 1) * NF],
                        in1=pT_bf[:, st, mc * NF:(mc + 1) * NF],
                    )

            # ---- softmax denominators: partition-reduce z_acc via ones-mm,
            # bounce [1, M] through DRAM to get [128, MT] columns, recip ----
            z_bf = persist.tile([P, M], BF16, name="z_bf")
            nc.vector.tensor_copy(z_bf[:], z_acc[:])
            z_row = persist.tile([1, M], F32, name="z_row")
            for mc in range(M // NF):
                ps_z = pp_z.tile([1, NF], F32, tag="z")
                nc.tensor.matmul(
                    ps_z[:],
                    lhsT=ones_bf[:],
                    rhs=z_bf[:, mc * NF:(mc + 1) * NF],
                    start=True,
                    stop=True,
                )
                nc.vector.tensor_copy(z_row[:, mc * NF:(mc + 1) * NF], ps_z[:])
            z_dram = dram.tile([1, M], F32, name="z_dram")
            nc.sync.dma_start(z_dram[:], z_row[:])
            z_col = persist.tile([P, MT], F32, name="z_col")
            nc.sync.dma_start(
                z_col[:], z_dram[0, :].rearrange("(t p) -> p t", p=P)
            )
            z_recip = persist.tile([P, MT], F32, name="z_recip")
            nc.vector.reciprocal(z_recip[:], z_col[:])

            # ---- O = (P^T)^T @ V, scaled by 1/Z ----
            for mt in range(MT):
                for ec in range(D // NF):
                    ps_o = pp_mm.tile([P, NF], F32, tag="mm")
                    for st in range(ST):
                        nc.tensor.matmul(
                            ps_o[:],
                            lhsT=pT_bf[:, st, mt * P:(mt + 1) * P],
                            rhs=v2[st // 8][:, st % 8, ec * NF:(ec + 1) * NF],
                            start=(st == 0),
                            stop=(st == ST - 1),
                        )
                    o_t = ostage.tile([P, NF], F32, tag="o")
                    nc.vector.tensor_scalar_mul(
                        o_t[:], ps_o[:], z_recip[:, mt:mt + 1]
                    )
                    nc.sync.dma_start(out_r[:, mt, ec * NF:(ec + 1) * NF], o_t[:])

    nc.compile()
    return nc


_nc_lock = threading.Lock()
_nc_cache = []


def _get_nc():
    with _nc_lock:
        if not _nc_cache:
            _nc_cache.append(build_attention_core())
        return _nc_cache[0]


def _make_in_maps(inputs, w_q, w_k, w_v):
    import ml_dtypes

    bf = ml_dtypes.bfloat16
    wqT = np.ascontiguousarray(np.asarray(w_q, dtype=np.float32).T).astype(bf)
    wkT = np.ascontiguousarray(np.asarray(w_k, dtype=np.float32).T).astype(bf)
    wvT = np.ascontiguousarray(np.asarray(w_v, dtype=np.float32).T).astype(bf)
    in_maps = []
    for core in range(8):
        b, half = core // 2, core % 2
        xq = np.asarray(inputs[b][half * M:(half + 1) * M, :], dtype=np.float32)
        in_maps.append(
            {
                "xq": np.ascontiguousarray(xq.T).astype(bf),
                "wqT": wqT,
                "wkT": wkT,
                "wvT": wvT,
            }
        )
    return in_maps


def run(inputs, w_q, w_k, w_v, **run_kwargs):
    """Run the 8-core SPMD kernel; returns (full_output, BassKernelResults)."""
    nc = _get_nc()
    in_maps = _make_in_maps(inputs, w_q, w_k, w_v)
    res = run_bass_kernel_spmd(nc, in_maps, core_ids=list(range(8)), **run_kwargs)
    full = np.empty((4, S, D), dtype=np.float32)
    for core in range(8):
        b, half = core // 2, core % 2
        full[b, half * M:(half + 1) * M, :] = res.results[core]["out"]
    return full, res


def kernel(**inputs) -> np.ndarray:
    out, _ = run(inputs["inputs"], inputs["w_q"], inputs["w_k"], inputs["w_v"])
    return out
